# revision 1
# baseline (speedup 1.0000x reference)
import numpy as np

M = 7
EPS = 1e-8


def _build_consts(H, Hh):
    n = np.arange(H)
    # ortho DFT matrix
    Fm = np.exp(-2j * np.pi * np.outer(n, n) / H) / np.sqrt(H)
    perm = (n + H // 2) % H  # fftshift row permutation (H even: ifftshift identical)
    A = Fm[perm, :]                     # A = P @ F   -> fftshift(fft(x)) = A x
    Binv = np.conj(Fm)[:, perm]         # Binv = conj(F) @ P -> ifft(ifftshift(z)) = Binv z
    # bilinear upsample matrix (align_corners=False, scale 2, edge-renormalized)
    U = np.zeros((H, Hh), np.float32)
    for i in range(H):
        c = (i + 0.5) / 2.0 - 0.5
        j0 = int(np.floor(c))
        w = c - j0
        j0c = min(max(j0, 0), Hh - 1)
        j1c = min(max(j0 + 1, 0), Hh - 1)
        U[i, j0c] += 1.0 - w
        U[i, j1c] += w
    return (
        A.real.astype(np.float32), A.imag.astype(np.float32),
        Binv.real.astype(np.float32), Binv.imag.astype(np.float32),
        U,
    )


def _make_fn(jnp, Ar, Ai, Br, Bi, U, W1, b1, W2, b2, refine_W, refine_b):
    lin = np.linspace(-(M // 2), M // 2, M).astype(np.float32)
    yg, xg = np.meshgrid(lin, lin, indexing="ij")
    yg = jnp.asarray(yg)
    xg = jnp.asarray(xg)

    def f(x_low_b, x_high_b):
        # x_low_b: (C,H,W), x_high_b: (C,Hh,Wh)
        C, H, W = x_low_b.shape
        # ---- forward FFT (shifted) of all channels: Y = A X A^T ----
        AX_r = jnp.einsum("ij,cjk->cik", Ar, x_low_b)
        AX_i = jnp.einsum("ij,cjk->cik", Ai, x_low_b)
        Yr = jnp.einsum("cik,jk->cij", AX_r, Ar) - jnp.einsum("cik,jk->cij", AX_i, Ai)
        Yi = jnp.einsum("cik,jk->cij", AX_r, Ai) + jnp.einsum("cik,jk->cij", AX_i, Ar)

        # ---- param net from center magnitude patch ----
        mag = jnp.sqrt(Yr * Yr + Yi * Yi)
        mag_w = jnp.mean(mag, axis=0)
        h0 = H // 2 - M // 2
        center = mag_w[h0:h0 + M, h0:h0 + M].reshape(M * M)
        hid = jnp.maximum(center @ W1.T + b1, 0.0)
        params = hid @ W2.T + b2

        # ---- anisotropic gaussian kernel (half-angle, no arctan2) ----
        a, b, p2 = params[0], params[1], params[2]
        r = jnp.sqrt(a * a + b * b)
        cos_t = b / r
        # t = atan2(a,b); theta = t/2 + pi/2 -> cos th = -sin(t/2), sin th = cos(t/2)
        sin_half = jnp.sign(a) * jnp.sqrt(jnp.maximum((1.0 - cos_t) / 2.0, 0.0))
        cos_half = jnp.sqrt(jnp.maximum((1.0 + cos_t) / 2.0, 0.0))
        cth = -sin_half
        sth = cos_half
        lam1 = jnp.exp(p2)
        lam2 = 1.0 / (lam1 + EPS)
        x_rot = xg * cth + yg * sth
        y_rot = -xg * sth + yg * cth
        k = jnp.exp(-(x_rot ** 2 / (2.0 * lam1 ** 2) + y_rot ** 2 / (2.0 * lam2 ** 2)))
        k = k / (jnp.sum(k) + EPS)

        # ---- depthwise 7x7 conv (zero pad) on freq maps via shift-sum ----
        pad = M // 2
        Yr_p = jnp.pad(Yr, ((0, 0), (pad, pad), (pad, pad)))
        Yi_p = jnp.pad(Yi, ((0, 0), (pad, pad), (pad, pad)))
        Zr = jnp.zeros_like(Yr)
        Zi = jnp.zeros_like(Yi)
        for aa in range(M):
            for bb in range(M):
                w = k[aa, bb]
                Zr = Zr + w * Yr_p[:, aa:aa + H, bb:bb + W]
                Zi = Zi + w * Yi_p[:, aa:aa + H, bb:bb + W]

        # ---- refine 1x1 conv over channels ----
        Zr = jnp.einsum("oc,chw->ohw", refine_W, Zr) + refine_b[:, None, None]
        Zi = jnp.einsum("oc,chw->ohw", refine_W, Zi) + refine_b[:, None, None]

        # ---- inverse: real(Binv Z Binv^T) ----
        T1r = jnp.einsum("ij,cjk->cik", Br, Zr) - jnp.einsum("ij,cjk->cik", Bi, Zi)
        T1i = jnp.einsum("ij,cjk->cik", Br, Zi) + jnp.einsum("ij,cjk->cik", Bi, Zr)
        out = jnp.einsum("cik,jk->cij", T1r, Br) - jnp.einsum("cik,jk->cij", T1i, Bi)

        # ---- bilinear upsample of x_high and fuse ----
        up = jnp.einsum("ij,cjk->cik", U, x_high_b)
        up = jnp.einsum("cik,jk->cij", up, U)
        return out + up

    return f


def kernel(**inputs):
    import jax
    import jax.numpy as jnp

    x_high = np.asarray(inputs["x_high"], np.float32)
    x_low = np.asarray(inputs["x_low"], np.float32)
    W1 = np.asarray(inputs["W1"], np.float32)
    b1 = np.asarray(inputs["b1"], np.float32)
    W2 = np.asarray(inputs["W2"], np.float32)
    b2 = np.asarray(inputs["b2"], np.float32)
    refine_W = np.asarray(inputs["refine_W"], np.float32)
    refine_b = np.asarray(inputs["refine_b"], np.float32)

    B, C, H, W = x_low.shape
    Hh = x_high.shape[2]
    Ar, Ai, Br, Bi, U = _build_consts(H, Hh)

    f = _make_fn(jnp, Ar, Ai, Br, Bi, U, W1, b1, W2, b2, refine_W, refine_b)

    out = None
    try:
        devs = jax.devices()
        if len(devs) >= B:
            pf = jax.pmap(f, devices=devs[:B])
            out = np.asarray(pf(x_low, x_high))
            if not np.isfinite(out).all():
                out = None
    except Exception:
        out = None

    if out is None:
        # CPU fallback
        with jax.default_device(jax.devices("cpu")[0]):
            jf = jax.jit(jax.vmap(f))
            out = np.asarray(jf(x_low, x_high))

    return out.astype(np.float32)
